# revision 87
# baseline (speedup 1.0000x reference)
"""Trainium2 Bass kernel for an AttentionBlock (GroupNorm + 8-head self-attn + proj + residual).

Sharding: data-parallel over batch. B=8 batch elements -> one per NeuronCore.
Each core runs an identical program on its own [C=512, T=1024] slice; the host
shards inputs / gathers outputs. No collectives.

Per-core pipeline (layouts partition-major, 128 partitions):
  x      [128p, 4ct, 1024t]   channels on partitions (fp32, kept for residual)
  GroupNorm in two independent halves (groups never span channel-tile pairs):
     free-dim sum on VectorE + Square-with-accumulator on ScalarE; group
     reduce/broadcast via tiny fp32 PE matmuls; rstd = exp(-0.5*ln(var+eps));
     affine on ScalarE -> xn in fp8e4 (the first half's affine unblocks the
     qk/vT DoubleRow pass u=0 while tiles 2,3 still stream in)
  qk   = Wqk @ xn + b  (fp8e4 DoubleRow matmuls, K=256/pass, fp32 psum,
         drained to bf16 by DVE)  rows = [q(512) | k(512)]
  vT   = xn^T @ WvT    (fp8 DoubleRow) stored per head-pair as
         [v_2j |1|1| v_2j+1] fp8e4 (ones columns make the AV matmul emit the
         softmax denominator Z replicated on the opposite partition half)
  lT[s,t] = k_h^T q_h per head: K=64 bf16 matmuls, 2 heads row-packed
  w    = exp(lT/8), no max-subtraction (logits/8 in [-7,8]) -> fp8e5m2:
         st 0-1 on the DVE via the Schraudolph bit trick (int8 bits of the
         fp8e5 value, dedicated 1-buf psum pool ps_c so the ScalarE exp
         rotation in ps_a is never DVE-coupled); st 2-7 exact exp on ScalarE
         with free fp8e5 output conversion
  a|Z  = [v|1]^T @ w : fp8 DoubleRow, 4 K=256 passes; per head the attention
         rows land on their final partition half, Z on the other
  1/Z  via DVE reciprocal_approx_fast at base partition 0 (custom-DVE ucode
         breaks at other bases); a 64-partition SBUF->SBUF DMA (issued from
         the idle GpSimd queue) does the cross-partition move; drain emission
         is two-phase so the strict-FIFO DVE never waits on in-flight DMAs
  out  = WpT^T @ a (fp8 DoubleRow) + bias seeded into psum by a K=1 PE
         matmul (bp x ones) + residual merged into the single DVE drain op

Schedule: one-pair software pipeline -- logits/exp of pair j+1 emitted before
AV of pair j; remaining qk rows + vT ride as fillers inside the logits
emission so their matmuls use psum-rotation idle gaps and their drains hit
the DVE queue in small doses; the last pair's AV accumulates into free ps_a
[P,T] tiles (FD=1024 drains, no n-serialization) with proj following n-outer.
DMA dispatch (~1.7us/queue-occupancy each) is spread across SP/ACT/GpSimd.

Engine budget (sim): ACT ~67us (exp-dominated), DVE ~62us, PE ~45us.
"""

import numpy as np

import concourse.bass as bass
import concourse.mybir as mybir
import concourse.tile as tile
from concourse import bacc
from contextlib import ExitStack

B = 8
C = 512
T = 1024
NH = 8            # heads
CH = 64           # channels per head
G = 32            # groups
CPG = C // G      # 16 channels per group
EPS = 1e-5
P = 128
NCT = C // P      # 4 channel tiles
NTT = T // P      # 8 sequence tiles
NQK = (2 * C) // P  # 8 row tiles of [q;k]
NC2 = T // 512    # 2 free-dim chunks of 512
HS = 2 * CH       # 128: per-head lhsT width in vT (64 v cols + 64 ones cols)

F32 = mybir.dt.float32
F32R = mybir.dt.float32r
BF16 = mybir.dt.bfloat16
FP8 = mybir.dt.float8e4
FP8E5 = mybir.dt.float8e5
I8 = mybir.dt.int8
FX = mybir.ActivationFunctionType
ALU = mybir.AluOpType
AX = mybir.AxisListType
DR = mybir.MatmulPerfMode.DoubleRow

# (pair, st, half) tiles whose softmax exp runs on the DVE via the Schraudolph
# bit trick (int8 bits of fp8e5m2 = round(x*log2e*4 + (60 - c))); the rest run
# exact exp on ScalarE with direct fp8e5 output. Balances the two engines.
def _exp_on_dve(j, st, half):
    # first st tiles: their serial fill->sch chain (dedicated 1-buf psum
    # pool ps_c) runs at the start of each pair window, overlapped with the
    # ScalarE exp stream for the rest (ps_a rotation, never DVE-coupled)
    return st in (0, 1)


SCH_A = 0.125 * 4.0 / np.log(2.0)
SCH_B = 60.0 - 0.2916


_DEBUG_TAP = None  # set by debug scripts before build_nc()
_STAGE = 5  # 1=GN/xn 2=+qkv/vT 3=+logits/exp 4=+AV 5=full (timing bisection)
_AV_MODE = "full"  # full | copy (no normalize) | local (no DMA, wrong nums)
_SMALL_BUFS = 6


def _build_body(ctx, tc, io):
    nc = tc.nc
    x_d = io["x"]
    out_d = io["out"]

    pers = ctx.enter_context(tc.tile_pool(name="pers", bufs=1))
    wt_pool = ctx.enter_context(tc.tile_pool(name="wt", bufs=3))
    small = ctx.enter_context(tc.tile_pool(name="small", bufs=_SMALL_BUFS))
    wide = ctx.enter_context(tc.tile_pool(name="wide", bufs=2))
    outp = ctx.enter_context(tc.tile_pool(name="outp", bufs=3))
    ps_a = ctx.enter_context(tc.tile_pool(name="ps_a", bufs=2, space="PSUM"))
    ps_b = ctx.enter_context(tc.tile_pool(name="ps_b", bufs=2, space="PSUM"))
    ps_c = ctx.enter_context(tc.tile_pool(name="ps_c", bufs=1, space="PSUM"))

    # ---- persistent SBUF tensors -------------------------------------------
    x_sb = pers.tile([P, NCT, T], F32, tag="x")
    xr_sb = pers.tile([P, NCT, T], F32, tag="xr")
    xn_sb = pers.tile([P, NCT, T], FP8, tag="xn")
    wqkT_sb = pers.tile([P, NCT, 2 * C], FP8, tag="wqkT")
    wvT_sb = pers.tile([P, NCT, C], FP8, tag="wvT")
    wpT_sb = pers.tile([P, NCT, C], FP8, tag="wpT")
    qk_sb = pers.tile([P, NQK, T], BF16, tag="qk")
    vT_sb = pers.tile([P, NTT, NH * HS], FP8, tag="vT")
    a_sb = pers.tile([P, NCT, T], FP8, tag="a")
    gw_sb = pers.tile([P, NCT], F32, tag="gw")
    gb_sb = pers.tile([P, NCT], F32, tag="gb")
    bqk_sb = pers.tile([P, NQK], F32, tag="bqk")
    bp_sb = pers.tile([P, NCT], F32, tag="bp")
    gmat_sb = pers.tile([P, 2, 16], F32, tag="gmat")
    gtmat_sb = pers.tile([16, 2, P], F32, tag="gtmat")
    stats_sb = pers.tile([P, NCT, 2], F32, tag="stats")
    ab_sb = pers.tile([P, NCT, 2], F32, tag="ab")
    xsq_sb = pers.tile([P, T], F32, tag="xsq")

    # ---- input DMAs ---------------------------------------------------------
    # Each dma_start serially occupies its issuing engine's queue for
    # ~1.7us of descriptor dispatch + transfer, so the inputs are spread
    # across all three DMA-capable queues (SP, ACT, GpSimd) ordered by when
    # each tensor is first needed.
    nc.sync.dma_start(x_sb[:, 0, :], x_d[0])
    nc.scalar.dma_start(x_sb[:, 1, :], x_d[1])
    nc.gpsimd.dma_start(x_sb[:, 2, :], x_d[2])
    # x3 before wqkT on SP: x3 gates the longer chain (g1 stats -> affine
    # tiles 2,3 -> qk pass u1 -> the drains that release the first logits)
    nc.sync.dma_start(x_sb[:, 3, :], x_d[3])
    nc.scalar.dma_start(gmat_sb[:], io["gmat"])
    nc.scalar.dma_start(gtmat_sb[:], io["gtmat"])
    nc.gpsimd.dma_start(gw_sb[:], io["gw"])
    nc.gpsimd.dma_start(gb_sb[:], io["gb"])
    nc.sync.dma_start(wqkT_sb[:], io["wqkT"])
    nc.gpsimd.dma_start(wvT_sb[:], io["wvT"])
    nc.sync.dma_start(bqk_sb[:], io["bqk"])
    nc.sync.dma_start(wpT_sb[:], io["wpT"])
    nc.sync.dma_start(bp_sb[:], io["bp"])

    # Per head-pair j, vT cols j*256..(j+1)*256 hold [v_2j | ones | ones | v_2j+1].
    # The AV lhsT for head h is cols h*128..(h+1)*128: [v|1] for even heads,
    # [1|v] for odd heads, so each head's attention rows land on the same
    # partition half as its final slot in a_sb, and the softmax denominator
    # lands replicated on the other half -- every consumer op stays
    # partition-aligned (HW engines cannot shift data across partitions).
    ones_view = vT_sb[:, :, : NH * HS].rearrange(
        "p s (pr i) -> p s pr i", i=2 * HS
    )[:, :, :, CH : CH + HS]
    nc.gpsimd.memset(ones_view, 1.0)
    # bias row for the proj-psum seed matmul + its ones rhs (K=1 PE matmul
    # seeds pr_ps with the output bias, freeing an ACT bias-add in the tail)
    bpT_sb = pers.tile([1, C], BF16, tag="bpT")
    onesr_sb = pers.tile([1, 512], BF16, tag="onesr")
    nc.scalar.dma_start(bpT_sb[:], io["bpT"])
    nc.gpsimd.memset(onesr_sb[:], 1.0)

    # ---- GroupNorm statistics ----------------------------------------------
    # Groups never span channel tiles (16 channels contiguous within a tile's
    # partitions), so GN runs as two independent halves over tiles {0,1} and
    # {2,3}: the first half's affine (and the qk/vT DoubleRow pass u=0, which
    # only reads xn tiles 0,1) starts while tiles 2,3 are still streaming in.
    # Per-channel sum on VectorE + sum-of-squares via ScalarE's fused
    # accumulator (both engines otherwise idle in the prologue).
    HG = 16  # groups per half
    for g in range(2):
        tj = (2 * g, 2 * g + 1)
        for jj, j in enumerate(tj):
            nc.vector.reduce_sum(stats_sb[:, j, 0:1], x_sb[:, j, :], axis=AX.X)
            nc.scalar.activation(
                xsq_sb[:], x_sb[:, j, :], FX.Square, accum_out=stats_sb[:, j, 1:2]
            )
        gstat_ps = ps_b.tile([P, 512], F32, tag="psb", name=f"gstat{g}")
        for jj, j in enumerate(tj):
            nc.tensor.matmul(
                gstat_ps[0:HG, 0:2],
                lhsT=gmat_sb[:, jj, :],
                rhs=stats_sb[:, j, :],
                start=(jj == 0),
                stop=(jj == 1),
            )
        mv = small.tile([HG, 2], F32, tag="mv")
        nc.vector.tensor_scalar_mul(mv[:], gstat_ps[0:HG, 0:2], 1.0 / (CPG * T))
        msq = small.tile([HG, 1], F32, tag="msq")
        nc.vector.tensor_mul(msq[:], mv[:, 0:1], mv[:, 0:1])
        veps = small.tile([HG, 1], F32, tag="veps")
        nc.vector.tensor_scalar(veps[:], msq[:], -1.0, EPS, ALU.mult, ALU.add)
        nc.vector.tensor_tensor(veps[:], mv[:, 1:2], veps[:], ALU.add)
        # rstd = exp(-0.5 * ln(var+eps)); Ln and Exp share one ACT table set
        # and are far more accurate than the Sqrt table (~2 ULP vs 65536)
        lnv = small.tile([HG, 1], F32, tag="lnv")
        nc.scalar.activation(lnv[:], veps[:], FX.Ln)
        stats2h = small.tile([HG, 2], F32, tag="stats2h", name=f"s2h{g}")
        nc.scalar.activation(stats2h[:, 1:2], lnv[:], FX.Exp, scale=-0.5)
        nc.vector.tensor_copy(stats2h[:, 0:1], mv[:, 0:1])

        # broadcast (mean, rstd) to channels; A = w*rstd, B = b - mean*A
        ab_ps = ps_b.tile([P, 512], F32, tag="psb", name=f"ab{g}")
        for jj in range(2):
            nc.tensor.matmul(
                ab_ps[:, 2 * jj : 2 * jj + 2],
                lhsT=gtmat_sb[0:HG, jj, :],
                rhs=stats2h[:],
                start=True,
                stop=True,
                skip_group_check=True,
            )
        mean_v = ab_ps[:, 0:4].rearrange("p (j two) -> p j two", two=2)[:, :, 0]
        rstd_v = ab_ps[:, 0:4].rearrange("p (j two) -> p j two", two=2)[:, :, 1]
        a_v = ab_sb[:, 2 * g : 2 * g + 2, 0]
        b_v = ab_sb[:, 2 * g : 2 * g + 2, 1]
        nc.vector.tensor_tensor(a_v, gw_sb[:, tj[0] : tj[1] + 1], rstd_v, ALU.mult)
        nc.vector.tensor_tensor(b_v, mean_v, a_v, ALU.mult)
        nc.vector.tensor_tensor(
            b_v, gb_sb[:, tj[0] : tj[1] + 1], b_v, ALU.subtract
        )
        for j in tj:
            nc.scalar.activation(
                xn_sb[:, j, :],
                x_sb[:, j, :],
                FX.Identity,
                bias=ab_sb[:, j, 1:2],
                scale=ab_sb[:, j, 0:1],
            )

    # Residual copy of x on the idle Pool engine: moves x_sb's LAST reader
    # from the proj drains (iteration end) to here, so the next For_i
    # iteration's x DMAs are not WAR-blocked behind this iteration's tail.
    for j in range(NCT):
        nc.gpsimd.tensor_copy(xr_sb[:, j, :], x_sb[:, j, :])

    # ---- qk = Wqk @ xn + b --------------------------------------------------
    def qk_unit(m, n):
        def emit():
            qk_ps = ps_b.tile([P, 512], F32, tag="psb")
            for u in range(NCT // 2):
                nc.tensor.matmul(
                    qk_ps[:],
                    lhsT=wqkT_sb[:, 2 * u : 2 * u + 2, m * P : (m + 1) * P],
                    rhs=xn_sb[:, 2 * u : 2 * u + 2, n * 512 : (n + 1) * 512],
                    start=(u == 0),
                    stop=(u == NCT // 2 - 1),
                    perf_mode=DR,
                )
            nc.vector.tensor_scalar_add(
                qk_sb[:, m, n * 512 : (n + 1) * 512], qk_ps[:], bqk_sb[:, m : m + 1]
            )

        return emit

    def emit_qk(m, chunks=range(NC2)):
        for n in chunks:
            qk_unit(m, n)()

    def vt_unit(st):
        return lambda: emit_vt(st)

    def emit_vt(st):
        vt_ps = ps_b.tile([P, 512], F32, tag="psb")
        for u in range(NCT // 2):
            nc.tensor.matmul(
                vt_ps[:],
                lhsT=xn_sb[:, 2 * u : 2 * u + 2, st * P : (st + 1) * P],
                rhs=wvT_sb[:, 2 * u : 2 * u + 2, :],
                start=(u == 0),
                stop=(u == NCT // 2 - 1),
                perf_mode=DR,
            )
        blk = vT_sb[:, st, : NH * HS].rearrange("p (pr i) -> p pr i", i=2 * HS)
        src = vt_ps[:].rearrange("p (pr half i) -> p pr half i", half=2, i=CH)
        nc.vector.tensor_copy(blk[:, :, 0:CH], src[:, :, 0, :])
        nc.vector.tensor_copy(blk[:, :, 2 * HS - CH : 2 * HS], src[:, :, 1, :])

    def emit_pair_logits(j, wt, fillers=()):
        """Logits + exp for head pair (2j, 2j+1); row-packed K=64 matmuls.
        exp tiles assigned to the DVE (Schraudolph fp8e5 bits) are NOT
        emitted here -- they're returned as a closure the pipeline calls
        after the previous pair's AV drain, so they never block the DVE FIFO
        while waiting on their logits (which would stall PSUM rotation and
        starve ScalarE). The rest run exact exp on ScalarE with direct fp8e5
        output. `fillers` are closures (qk/vT units) emitted one per (st,
        half) iteration from st=1 on -- their PE matmuls slot into the
        psum-rotation idle gaps of the logits stream without delaying it,
        and their drains dribble into the DVE queue in small doses."""
        fillers = list(fillers)
        for st in range(NTT):
            for half in range(2):
                on_dve = _exp_on_dve(j, st, half)
                pool = ps_c if on_dve else ps_a
                lt = pool.tile([P, T], F32, tag="psc" if on_dve else "psa")
                rs = slice(half * CH, (half + 1) * CH)
                for n in range(NC2):
                    nc.tensor.matmul(
                        lt[:, n * 512 : (n + 1) * 512],
                        lhsT=qk_sb[rs, 4 + j, st * P : (st + 1) * P],
                        rhs=qk_sb[rs, j, n * 512 : (n + 1) * 512],
                        start=True,
                        stop=True,
                    )
                wslot = wt[:, st, half * T : (half + 1) * T]
                if on_dve:
                    nc.vector.tensor_scalar(
                        wslot.bitcast(I8), lt[:], SCH_A, SCH_B, ALU.mult, ALU.add
                    )
                else:
                    nc.scalar.activation(wslot, lt[:], FX.Exp, scale=0.125)
                if st >= 2 and fillers:
                    fillers.pop(0)()

    def _proj_mms(pr_ps, m, n):
        # bias seed via K=1 matmul (bp x ones), then fp8 DoubleRow passes
        nc.tensor.matmul(
            pr_ps[:],
            lhsT=bpT_sb[0:1, m * P : (m + 1) * P],
            rhs=onesr_sb[0:1, :],
            start=True,
            stop=False,
            skip_group_check=True,
        )
        for u in range(NCT // 2):
            nc.tensor.matmul(
                pr_ps[:],
                lhsT=wpT_sb[:, 2 * u : 2 * u + 2, m * P : (m + 1) * P],
                rhs=a_sb[:, 2 * u : 2 * u + 2, n * 512 : (n + 1) * 512],
                start=False,
                stop=(u == NCT // 2 - 1),
                perf_mode=DR,
                skip_group_check=True,
            )

    def emit_proj_all():
        # a_sb is fully drained when this runs (the wide AV writes whole-T
        # rows), so both n-chunks are ready at once. Odd m borrows the
        # tail-idle ps_c [P,T] tile for BOTH chunks: one FD=1024 drain and
        # one output DMA instead of two each; even m stays on ps_b [P,512]
        # per chunk -- 3 effective psum buffers deepen the tail pipeline.
        for m in range(NCT):
            # wide units drain on the (tail-idle) ACT queue, small on SP
            eng = nc.scalar if m % 2 == 1 else nc.sync
            if m % 2 == 1:
                pr_wide = ps_c.tile([P, T], F32, tag="psc", name=f"prc{m}")
                for n in range(NC2):
                    _proj_mms(pr_wide[:, n * 512 : (n + 1) * 512], m, n)
                ot = outp.tile([P, T], F32, tag="otw", name=f"otw{m}")
                nc.vector.tensor_add(ot[:], pr_wide[:], xr_sb[:, m, :])
                eng.dma_start(out_d[:, m, :], ot[:])
            else:
                for n in range(NC2):
                    pr_ps = ps_b.tile([P, 512], F32, tag="psb")
                    _proj_mms(pr_ps, m, n)
                    ot = outp.tile([P, 512], F32, tag="ot")
                    nc.vector.tensor_add(
                        ot[:], pr_ps[:], xr_sb[:, m, n * 512 : (n + 1) * 512]
                    )
                    eng.dma_start(
                        out_d[:, m, n * 512 : (n + 1) * 512], ot[:]
                    )

    def emit_pair_av_wide(j, wt):
        # tail variant: after the last exp the ps_a banks are free, so each
        # half accumulates both n-chunks into one [P, T] ps_a tile and the
        # drain runs at FD=1024 with no n-serialization; ps_b stays free for
        # the proj that follows.
        lo, hi = slice(0, CH), slice(CH, P)
        avs = {}
        for half in (1, 0):
            h = 2 * j + half
            av_ps = ps_a.tile([P, T], F32, tag="psa")
            for n in range(NC2):
                for u in range(NTT // 2):
                    nc.tensor.matmul(
                        av_ps[:, n * 512 : (n + 1) * 512],
                        lhsT=vT_sb[:, 2 * u : 2 * u + 2, h * HS : (h + 1) * HS],
                        rhs=wt[
                            :,
                            2 * u : 2 * u + 2,
                            half * T + n * 512 : half * T + (n + 1) * 512,
                        ],
                        start=(u == 0),
                        stop=(u == NTT // 2 - 1),
                        perf_mode=DR,
                        skip_group_check=True,
                    )
            avs[half] = av_ps
        zs1 = wide.tile([P, T], F32, tag="zsw")
        zi1 = wide.tile([P, T], F32, tag="ziw")
        nc.vector.reciprocal_approx_fast(zi1[lo, :], avs[1][lo, :])
        nc.gpsimd.dma_start(zs1[hi, :], zi1[lo, :])
        zs0 = wide.tile([P, T], F32, tag="zsw")
        zc0 = wide.tile([P, T], F32, tag="zcw")
        nc.vector.tensor_copy(zc0[hi, :], avs[0][hi, :])
        nc.gpsimd.dma_start(zs0[lo, :], zc0[hi, :])
        nc.vector.tensor_tensor(a_sb[hi, j, :], avs[1][hi, :], zs1[hi, :], ALU.mult)
        zi0 = wide.tile([P, T], F32, tag="ziw")
        nc.vector.reciprocal_approx_fast(zi0[lo, :], zs0[lo, :])
        nc.vector.tensor_tensor(a_sb[lo, j, :], avs[0][lo, :], zi0[lo, :], ALU.mult)

    def av_units(j, wt):
        # even head: rows 0..63 = attention, 64..127 = Z (odd: swapped). The
        # custom-DVE reciprocal only works at base partition 0, and engines
        # can't shift data across partitions -- a small SBUF->SBUF DMA does
        # the 64-partition move. Returns 6 closures per pair, interleaved as
        # fillers into the NEXT pair's logits emission so the AV matmuls run
        # in the PE's psum-rotation idle gaps (not after the last exp) and
        # the strict-FIFO DVE never waits on an in-flight shift DMA:
        # A(half, n) = matmuls + DMA-independent op + DMA kick;
        # B(n) = DMA-dependent recip/mults, >=2 filler slots later.
        lo, hi = slice(0, CH), slice(CH, P)
        avs, zss = {}, {}

        def unit_a(half, n):
            def emit():
                h = 2 * j + half
                av_ps = ps_b.tile([P, 512], F32, tag="psb")
                for u in range(NTT // 2):
                    nc.tensor.matmul(
                        av_ps[:],
                        lhsT=vT_sb[:, 2 * u : 2 * u + 2, h * HS : (h + 1) * HS],
                        rhs=wt[
                            :,
                            2 * u : 2 * u + 2,
                            half * T + n * 512 : half * T + (n + 1) * 512,
                        ],
                        start=(u == 0),
                        stop=(u == NTT // 2 - 1),
                        perf_mode=DR,
                    )
                avs[half, n] = av_ps
                zs = small.tile([P, 512], F32, tag="zs")
                zss[half, n] = zs
                if half == 1:
                    zi1 = small.tile([P, 512], F32, tag="zi")
                    nc.vector.reciprocal_approx_fast(zi1[lo, :], av_ps[lo, :])
                    nc.gpsimd.dma_start(zs[hi, :], zi1[lo, :])
                else:
                    zc0 = small.tile([P, 512], F32, tag="zc")
                    nc.vector.tensor_copy(zc0[hi, :], av_ps[hi, :])
                    nc.gpsimd.dma_start(zs[lo, :], zc0[hi, :])

            return emit

        def unit_b(n):
            def emit():
                nc.vector.tensor_tensor(
                    a_sb[hi, j, n * 512 : (n + 1) * 512],
                    avs[1, n][hi, :],
                    zss[1, n][hi, :],
                    ALU.mult,
                )
                zi0 = small.tile([P, 512], F32, tag="zi")
                nc.vector.reciprocal_approx_fast(zi0[lo, :], zss[0, n][lo, :])
                nc.vector.tensor_tensor(
                    a_sb[lo, j, n * 512 : (n + 1) * 512],
                    avs[0, n][lo, :],
                    zi0[lo, :],
                    ALU.mult,
                )

            return emit

        return [unit_a(1, 0), unit_a(0, 0), unit_b(0),
                unit_a(1, 1), unit_a(0, 1), unit_b(1)]

    def emit_pair_av(j, wt):
        for f in av_units(j, wt):
            f()


    def tap_out(src3d):
        for m in range(NCT):
            dt_t = outp.tile([P, T], F32, tag="dbg", name=f"stg{m}")
            nc.vector.tensor_copy(dt_t[:], src3d[:, m, :])
            nc.sync.dma_start(out_d[:, m, :], dt_t[:])

    if _STAGE == 1:
        tap_out(xn_sb)
        return

    # rows 0/4 up front (logits(0) needs them); the remaining qk rows and all
    # vT ride as fillers inside the pair windows: row j+1/5+j inside window j
    # so each pair's logits only ever wait on long-drained rows
    emit_qk(0)
    emit_qk(4)
    pad = lambda: None

    def window_fillers(av, qks):
        # [A,A,pad,B, A,A,pad,B, qk...]: respects the 2-buffer ps_b WAR
        # chain (an A-unit's psum buffer is reused 2 allocs later, so its
        # reader B sits in between) and gives every shift DMA >=2 slots
        # before its dependent B op.
        if av is None:
            return qks
        return [av[0], av[1], pad, av[2], av[3], av[4], pad, av[5]] + qks

    winf0 = (
        [qk_unit(1, n) for n in range(NC2)]
        + [qk_unit(5, n) for n in range(NC2)]
        + [vt_unit(st) for st in range(NTT)]
    )
    wts0 = wt_pool.tile([P, NTT, 2 * T], FP8E5, tag="w", name="wt0")
    emit_pair_logits(0, wts0, winf0)
    if _DEBUG_TAP == "w0":
        for m in range(NCT):
            dt_t = outp.tile([P, T], F32, tag="dbg", name=f"dbg{m}")
            nc.vector.tensor_copy(dt_t[:], wts0[:, m, 0:T])
            nc.sync.dma_start(out_d[:, m, :], dt_t[:])
        return
    if _DEBUG_TAP == "av0":
        emit_pair_av(0, wts0)
        return
    if _STAGE == 2:
        tap_out(qk_sb[:, 0:NCT, :])
        return
    if _STAGE == 3:
        # logits+exp only: tiny reads release the wt slots without AV
        dmy = small.tile([P, 8], F32, tag="dmy")
        nc.vector.tensor_copy(dmy[:, 0:8], wts0[:, 7, 2040:2048])
        for j in range(1, 4):
            wts = wt_pool.tile([P, NTT, 2 * T], FP8E5, tag="w", name=f"wt{j}")
            emit_pair_logits(j, wts)
            nc.vector.tensor_copy(dmy[:, 0:8], wts[:, 7, 2040:2048])
        tap_out(qk_sb[:, 0:NCT, :])
        return
    # One-pair software pipeline: logits(j+1) are emitted (= higher scheduler
    # priority) before AV(j), so ScalarE always has exp input ready while the
    # PE drains the previous pair's AV matmuls in the gaps. The remaining
    # qk row tiles are spread across the pair windows to keep the PE from
    # idling long enough to trip the HAM clock throttle.
    prev = wts0
    for j in range(1, 4):
        wts = wt_pool.tile([P, NTT, 2 * T], FP8E5, tag="w", name=f"wt{j}")
        qks = (
            [qk_unit(j + 1, n) for n in range(NC2)]
            + [qk_unit(5 + j, n) for n in range(NC2)]
            if j < 3
            else []
        )
        emit_pair_logits(j, wts, qks)
        emit_pair_av(j - 1, prev)
        prev = wts
    emit_pair_av_wide(3, prev)
    if _STAGE == 4:
        tap_out(a_sb)
        return

    # optional debug tap: overwrite `out` with an intermediate tensor
    if _DEBUG_TAP == "xn":
        for m in range(NCT):
            dt_t = outp.tile([P, T], F32, tag="dbg", name=f"dbg{m}")
            nc.vector.tensor_copy(dt_t[:], xn_sb[:, m, :])
            nc.sync.dma_start(out_d[:, m, :], dt_t[:])
        return
    if _DEBUG_TAP == "qk":  # q rows: qk tiles 0..3 -> out tiles 0..3
        for m in range(NCT):
            dt_t = outp.tile([P, T], F32, tag="dbg", name=f"dbg{m}")
            nc.vector.tensor_copy(dt_t[:], qk_sb[:, m, :])
            nc.sync.dma_start(out_d[:, m, :], dt_t[:])
        return
    if _DEBUG_TAP == "k":  # k rows: qk tiles 4..7
        for m in range(NCT):
            dt_t = outp.tile([P, T], F32, tag="dbg", name=f"dbg{m}")
            nc.vector.tensor_copy(dt_t[:], qk_sb[:, 4 + m, :])
            nc.sync.dma_start(out_d[:, m, :], dt_t[:])
        return
    if _DEBUG_TAP == "vt":  # vT tiles 0..3 (with ones cols)
        for m in range(NCT):
            dt_t = outp.tile([P, T], F32, tag="dbg", name=f"dbg{m}")
            nc.vector.tensor_copy(dt_t[:], vT_sb[:, m, :])
            nc.sync.dma_start(out_d[:, m, :], dt_t[:])
        return
    if _DEBUG_TAP == "a":
        for m in range(NCT):
            dt_t = outp.tile([P, T], F32, tag="dbg", name=f"dbg{m}")
            nc.vector.tensor_copy(dt_t[:], a_sb[:, m, :])
            nc.sync.dma_start(out_d[:, m, :], dt_t[:])
        return

    # ---- proj + bias + residual --------------------------------------------
    emit_proj_all()


_ACT_SET = "natural_log_exp_and_others"  # contains Square/Ln/Exp/Identity/Copy


def _pin_act_tables():
    """Restrict the ACT table chooser to one set that covers every function
    this kernel uses. Without this, bacc picks each function's 'home' set and
    the GroupNorm Square/Ln/Exp/Identity sequence thrashes ACT_TABLE_LOADs
    (~2.7us each on HW, 5 loads observed). Dict length/order is preserved so
    act_func_set_ids still index act_info.json correctly."""
    import concourse.bacc as _bacc

    orig = _bacc.get_activation_tables

    def patched(arch):
        tabs = orig(arch)
        return {k: (v if k == _ACT_SET else set()) for k, v in tabs.items()}

    _bacc.get_activation_tables = patched
    return lambda: setattr(_bacc, "get_activation_tables", orig)


def build_nc(loop_n=0):
    """loop_n > 0 wraps the body in a For_i running it loop_n times --
    used only by the timing harness to amortize host/RPC overhead."""
    restore = _pin_act_tables()
    try:
        return _build_nc_inner(loop_n)
    finally:
        restore()


def _build_nc_inner(loop_n=0):
    nc = bacc.Bacc("TRN2", target_bir_lowering=False, debug=False)
    io = {}
    io["x"] = nc.dram_tensor("x", [NCT, P, T], F32, kind="ExternalInput").ap()
    io["gw"] = nc.dram_tensor("gw", [P, NCT], F32, kind="ExternalInput").ap()
    io["gb"] = nc.dram_tensor("gb", [P, NCT], F32, kind="ExternalInput").ap()
    io["wqkT"] = nc.dram_tensor("wqkT", [P, NCT, 2 * C], FP8, kind="ExternalInput").ap()
    io["bqk"] = nc.dram_tensor("bqk", [P, NQK], F32, kind="ExternalInput").ap()
    io["wvT"] = nc.dram_tensor("wvT", [P, NCT, C], FP8, kind="ExternalInput").ap()
    io["wpT"] = nc.dram_tensor("wpT", [P, NCT, C], FP8, kind="ExternalInput").ap()
    io["bp"] = nc.dram_tensor("bp", [P, NCT], F32, kind="ExternalInput").ap()
    io["gmat"] = nc.dram_tensor("gmat", [P, 2, 16], F32, kind="ExternalInput").ap()
    io["gtmat"] = nc.dram_tensor("gtmat", [16, 2, P], F32, kind="ExternalInput").ap()
    io["bpT"] = nc.dram_tensor("bpT", [1, C], BF16, kind="ExternalInput").ap()
    io["out"] = nc.dram_tensor("out", [P, NCT, T], F32, kind="ExternalOutput").ap()
    with tile.TileContext(nc) as tc:
        with ExitStack() as ctx:
            if loop_n:
                with tc.For_i(0, loop_n, 1):
                    _build_body(ctx, tc, io)
            else:
                _build_body(ctx, tc, io)
    nc.compile()
    return nc


def _tile_cmaj(a, ntiles):
    """[ntiles*128, F...] -> [128, ntiles, F...] (partition-major tiling)."""
    return np.ascontiguousarray(
        a.reshape(ntiles, P, *a.shape[1:]).swapaxes(0, 1)
    )


def prep_inputs(x, norm_w, norm_b, qkv_w, qkv_b, proj_w, proj_b):
    f = np.float32
    x = np.asarray(x, f)
    norm_w = np.asarray(norm_w, f)
    norm_b = np.asarray(norm_b, f)
    qkv_w = np.asarray(qkv_w, f)
    qkv_b = np.asarray(qkv_b, f)
    proj_w = np.asarray(proj_w, f)
    proj_b = np.asarray(proj_b, f)

    wr = qkv_w.reshape(NH, 3, CH, C)
    Wq = wr[:, 0].reshape(C, C)
    Wk = wr[:, 1].reshape(C, C)
    Wv = wr[:, 2].reshape(C, C)
    br = qkv_b.reshape(NH, 3, CH)
    bq = br[:, 0].reshape(C)
    bk = br[:, 1].reshape(C)
    bv = br[:, 2].reshape(C)

    common = {}
    common["gw"] = _tile_cmaj(norm_w, NCT)
    common["gb"] = _tile_cmaj(norm_b, NCT)
    import concourse.mybir as _mb
    f8 = _mb.dt.np(FP8)

    def to_f8(a):
        return np.clip(a, -240.0, 240.0).astype(f8)

    common["wqkT"] = to_f8(_tile_cmaj(np.concatenate([Wq, Wk], 0).T.copy(), NCT))
    common["bqk"] = _tile_cmaj(np.concatenate([bq, bk]), NQK)
    common["wvT"] = to_f8(_tile_cmaj(Wv.T.copy(), NCT))
    common["wpT"] = to_f8(_tile_cmaj(proj_w.T.copy(), NCT))
    bp_full = proj_b + proj_w @ bv
    common["bp"] = _tile_cmaj(bp_full, NCT)
    import ml_dtypes
    common["bpT"] = bp_full.reshape(1, C).astype(ml_dtypes.bfloat16)

    # per-half group matrices: within a half, tile jj's partitions map to
    # groups 8*jj + p//16 (identical for both halves)
    pidx = np.arange(P)
    gmat = np.zeros((P, 2, 16), f)
    gtmat = np.zeros((16, 2, P), f)
    for jj in range(2):
        grp = 8 * jj + pidx // CPG
        gmat[pidx, jj, grp] = 1.0
        gtmat[grp, jj, pidx] = 1.0
    common["gmat"] = gmat
    common["gtmat"] = gtmat

    in_maps = []
    for b in range(B):
        m = dict(common)
        m["x"] = np.ascontiguousarray(x[b].reshape(NCT, P, T))
        in_maps.append(m)
    return in_maps


_NC_CACHE = []


def _get_nc():
    if not _NC_CACHE:
        _NC_CACHE.append(build_nc())
    return _NC_CACHE[0]


def run(in_maps, trace=False, **kw):
    from concourse.bass_utils import run_bass_kernel_spmd

    nc = _get_nc()
    return run_bass_kernel_spmd(nc, in_maps, list(range(B)), trace=trace, **kw)


def kernel(x, norm_w, norm_b, qkv_w, qkv_b, proj_w, proj_b):
    in_maps = prep_inputs(x, norm_w, norm_b, qkv_w, qkv_b, proj_w, proj_b)
    res = run(in_maps).results
    outs = [
        res[b]["out"].swapaxes(0, 1).reshape(C, 32, 32) for b in range(B)
    ]
    return np.stack(outs).astype(np.float32)


if __name__ == "__main__":
    nc = build_nc()
    print("built ok:", len(nc.m.functions[0].instructions) if hasattr(nc.m.functions[0], "instructions") else "n/a")



# revision 91
# speedup vs baseline: 1.0709x; 1.0709x over previous
"""Trainium2 Bass kernel for an AttentionBlock (GroupNorm + 8-head self-attn + proj + residual).

Sharding: data-parallel over batch. B=8 batch elements -> one per NeuronCore.
Each core runs an identical program on its own [C=512, T=1024] slice; the host
shards inputs / gathers outputs. No collectives.

Per-core pipeline (layouts partition-major, 128 partitions):
  x      [128p, 4ct, 1024t]   channels on partitions (fp32, kept for residual)
  GroupNorm in two independent halves (groups never span channel-tile pairs):
     free-dim sum on VectorE + Square-with-accumulator on ScalarE; group
     reduce/broadcast via tiny fp32 PE matmuls; rstd = exp(-0.5*ln(var+eps));
     affine on ScalarE -> xn in fp8e4 (the first half's affine unblocks the
     qk/vT DoubleRow pass u=0 while tiles 2,3 still stream in)
  qk   = Wqk @ xn + b  (fp8e4 DoubleRow matmuls, K=256/pass, fp32 psum,
         drained to bf16 by DVE)  rows = [q(512) | k(512)]
  vT   = xn^T @ WvT    (fp8 DoubleRow) stored per head-pair as
         [v_2j |1|1| v_2j+1] fp8e4 (ones columns make the AV matmul emit the
         softmax denominator Z replicated on the opposite partition half)
  lT[s,t] = k_h^T q_h per head: K=64 bf16 matmuls, 2 heads row-packed
  w    = exp(lT/8), no max-subtraction (logits/8 in [-7,8]) -> fp8e5m2:
         st 0-1 on the DVE via the Schraudolph bit trick (int8 bits of the
         fp8e5 value, dedicated 1-buf psum pool ps_c so the ScalarE exp
         rotation in ps_a is never DVE-coupled); st 2-7 exact exp on ScalarE
         with free fp8e5 output conversion
  a|Z  = [v|1]^T @ w : fp8 DoubleRow, 4 K=256 passes; per head the attention
         rows land on their final partition half, Z on the other
  1/Z  via DVE reciprocal_approx_fast at base partition 0 (custom-DVE ucode
         breaks at other bases); a 64-partition SBUF->SBUF DMA (issued from
         the idle GpSimd queue) does the cross-partition move; drain emission
         is two-phase so the strict-FIFO DVE never waits on in-flight DMAs
  out  = WpT^T @ a (fp8 DoubleRow) + bias seeded into psum by a K=1 PE
         matmul (bp x ones) + residual merged into the single DVE drain op

Schedule: one-pair software pipeline -- logits/exp of pair j+1 emitted before
AV of pair j; remaining qk rows + vT ride as fillers inside the logits
emission so their matmuls use psum-rotation idle gaps and their drains hit
the DVE queue in small doses; the last pair's AV accumulates into free ps_a
[P,T] tiles (FD=1024 drains, no n-serialization) with proj following n-outer.
DMA dispatch (~1.7us/queue-occupancy each) is spread across SP/ACT/GpSimd.

Engine budget (sim): ACT ~67us (exp-dominated), DVE ~62us, PE ~45us.
"""

import numpy as np

import concourse.bass as bass
import concourse.mybir as mybir
import concourse.tile as tile
from concourse import bacc
from contextlib import ExitStack

B = 8
C = 512
T = 1024
NH = 8            # heads
CH = 64           # channels per head
G = 32            # groups
CPG = C // G      # 16 channels per group
EPS = 1e-5
P = 128
NCT = C // P      # 4 channel tiles
NTT = T // P      # 8 sequence tiles
NQK = (2 * C) // P  # 8 row tiles of [q;k]
NC2 = T // 512    # 2 free-dim chunks of 512
HS = 2 * CH       # 128: per-head lhsT width in vT (64 v cols + 64 ones cols)

F32 = mybir.dt.float32
F32R = mybir.dt.float32r
BF16 = mybir.dt.bfloat16
FP8 = mybir.dt.float8e4
FP8E5 = mybir.dt.float8e5
I8 = mybir.dt.int8
FX = mybir.ActivationFunctionType
ALU = mybir.AluOpType
AX = mybir.AxisListType
DR = mybir.MatmulPerfMode.DoubleRow

# (pair, st, half) tiles whose softmax exp runs on the DVE via the Schraudolph
# bit trick (int8 bits of fp8e5m2 = round(x*log2e*4 + (60 - c))); the rest run
# exact exp on ScalarE with direct fp8e5 output. Balances the two engines.
def _exp_on_dve(j, st, half):
    # first st tiles: their serial fill->sch chain (dedicated 1-buf psum
    # pool ps_c) runs at the start of each pair window, overlapped with the
    # ScalarE exp stream for the rest (ps_a rotation, never DVE-coupled)
    return st in (0, 1)


SCH_A = 0.125 * 4.0 / np.log(2.0)
SCH_B = 60.0 - 0.2916


_DEBUG_TAP = None  # set by debug scripts before build_nc()
_STAGE = 5  # 1=GN/xn 2=+qkv/vT 3=+logits/exp 4=+AV 5=full (timing bisection)
_AV_MODE = "full"  # full | copy (no normalize) | local (no DMA, wrong nums)
_SMALL_BUFS = 6


def _build_body(ctx, tc, io):
    nc = tc.nc
    x_d = io["x"]
    out_d = io["out"]

    pers = ctx.enter_context(tc.tile_pool(name="pers", bufs=1))
    wt_pool = ctx.enter_context(tc.tile_pool(name="wt", bufs=3))
    small = ctx.enter_context(tc.tile_pool(name="small", bufs=_SMALL_BUFS))
    wide = ctx.enter_context(tc.tile_pool(name="wide", bufs=2))
    outp = ctx.enter_context(tc.tile_pool(name="outp", bufs=3))
    ps_a = ctx.enter_context(tc.tile_pool(name="ps_a", bufs=2, space="PSUM"))
    ps_b = ctx.enter_context(tc.tile_pool(name="ps_b", bufs=2, space="PSUM"))
    ps_c = ctx.enter_context(tc.tile_pool(name="ps_c", bufs=1, space="PSUM"))

    # ---- persistent SBUF tensors -------------------------------------------
    x_sb = pers.tile([P, NCT, T], F32, tag="x")
    xn_sb = pers.tile([P, NCT, T], FP8, tag="xn")
    wqkT_sb = pers.tile([P, NCT, 2 * C], FP8, tag="wqkT")
    wvT_sb = pers.tile([P, NCT, C], FP8, tag="wvT")
    wpT_sb = pers.tile([P, NCT, C], FP8, tag="wpT")
    qk_sb = pers.tile([P, NQK, T], BF16, tag="qk")
    vT_sb = pers.tile([P, NTT, NH * HS], FP8, tag="vT")
    a_sb = pers.tile([P, NCT, T], FP8, tag="a")
    gw_sb = pers.tile([P, NCT], F32, tag="gw")
    gb_sb = pers.tile([P, NCT], F32, tag="gb")
    bqk_sb = pers.tile([P, NQK], F32, tag="bqk")
    bp_sb = pers.tile([P, NCT], F32, tag="bp")
    gmat_sb = pers.tile([P, 2, 16], F32, tag="gmat")
    gtmat_sb = pers.tile([16, 2, P], F32, tag="gtmat")
    stats_sb = pers.tile([P, NCT, 2], F32, tag="stats")
    ab_sb = pers.tile([P, NCT, 2], F32, tag="ab")
    xsq_sb = pers.tile([P, T], F32, tag="xsq")

    # ---- input DMAs ---------------------------------------------------------
    # Each dma_start serially occupies its issuing engine's queue for
    # ~1.7us of descriptor dispatch + transfer, so the inputs are spread
    # across all three DMA-capable queues (SP, ACT, GpSimd) ordered by when
    # each tensor is first needed.
    nc.sync.dma_start(x_sb[:, 0, :], x_d[0])
    nc.scalar.dma_start(x_sb[:, 1, :], x_d[1])
    nc.gpsimd.dma_start(x_sb[:, 2, :], x_d[2])
    # x3 before wqkT on SP: x3 gates the longer chain (g1 stats -> affine
    # tiles 2,3 -> qk pass u1 -> the drains that release the first logits)
    nc.sync.dma_start(x_sb[:, 3, :], x_d[3])
    nc.scalar.dma_start(gmat_sb[:], io["gmat"])
    nc.scalar.dma_start(gtmat_sb[:], io["gtmat"])
    nc.gpsimd.dma_start(gw_sb[:], io["gw"])
    nc.gpsimd.dma_start(gb_sb[:], io["gb"])
    nc.sync.dma_start(wqkT_sb[:], io["wqkT"])
    nc.gpsimd.dma_start(wvT_sb[:], io["wvT"])
    nc.sync.dma_start(bqk_sb[:], io["bqk"])
    nc.sync.dma_start(wpT_sb[:], io["wpT"])
    nc.sync.dma_start(bp_sb[:], io["bp"])

    # Per head-pair j, vT cols j*256..(j+1)*256 hold [v_2j | ones | ones | v_2j+1].
    # The AV lhsT for head h is cols h*128..(h+1)*128: [v|1] for even heads,
    # [1|v] for odd heads, so each head's attention rows land on the same
    # partition half as its final slot in a_sb, and the softmax denominator
    # lands replicated on the other half -- every consumer op stays
    # partition-aligned (HW engines cannot shift data across partitions).
    ones_view = vT_sb[:, :, : NH * HS].rearrange(
        "p s (pr i) -> p s pr i", i=2 * HS
    )[:, :, :, CH : CH + HS]
    nc.gpsimd.memset(ones_view, 1.0)
    # bias row for the proj-psum seed matmul + its ones rhs (K=1 PE matmul
    # seeds pr_ps with the output bias, freeing an ACT bias-add in the tail)
    bpT_sb = pers.tile([1, C], BF16, tag="bpT")
    onesr_sb = pers.tile([1, 512], BF16, tag="onesr")
    nc.scalar.dma_start(bpT_sb[:], io["bpT"])
    nc.gpsimd.memset(onesr_sb[:], 1.0)

    # ---- GroupNorm statistics ----------------------------------------------
    # Groups never span channel tiles (16 channels contiguous within a tile's
    # partitions), so GN runs as two independent halves over tiles {0,1} and
    # {2,3}: the first half's affine (and the qk/vT DoubleRow pass u=0, which
    # only reads xn tiles 0,1) starts while tiles 2,3 are still streaming in.
    # Per-channel sum on VectorE + sum-of-squares via ScalarE's fused
    # accumulator (both engines otherwise idle in the prologue).
    HG = 16  # groups per half
    for g in range(2):
        tj = (2 * g, 2 * g + 1)
        for jj, j in enumerate(tj):
            nc.vector.reduce_sum(stats_sb[:, j, 0:1], x_sb[:, j, :], axis=AX.X)
            nc.scalar.activation(
                xsq_sb[:], x_sb[:, j, :], FX.Square, accum_out=stats_sb[:, j, 1:2]
            )
        gstat_ps = ps_b.tile([P, 512], F32, tag="psb", name=f"gstat{g}")
        for jj, j in enumerate(tj):
            nc.tensor.matmul(
                gstat_ps[0:HG, 0:2],
                lhsT=gmat_sb[:, jj, :],
                rhs=stats_sb[:, j, :],
                start=(jj == 0),
                stop=(jj == 1),
            )
        mv = small.tile([HG, 2], F32, tag="mv")
        nc.vector.tensor_scalar_mul(mv[:], gstat_ps[0:HG, 0:2], 1.0 / (CPG * T))
        msq = small.tile([HG, 1], F32, tag="msq")
        nc.vector.tensor_mul(msq[:], mv[:, 0:1], mv[:, 0:1])
        veps = small.tile([HG, 1], F32, tag="veps")
        nc.vector.tensor_scalar(veps[:], msq[:], -1.0, EPS, ALU.mult, ALU.add)
        nc.vector.tensor_tensor(veps[:], mv[:, 1:2], veps[:], ALU.add)
        # rstd = exp(-0.5 * ln(var+eps)); Ln and Exp share one ACT table set
        # and are far more accurate than the Sqrt table (~2 ULP vs 65536)
        lnv = small.tile([HG, 1], F32, tag="lnv")
        nc.scalar.activation(lnv[:], veps[:], FX.Ln)
        stats2h = small.tile([HG, 2], F32, tag="stats2h", name=f"s2h{g}")
        nc.scalar.activation(stats2h[:, 1:2], lnv[:], FX.Exp, scale=-0.5)
        nc.vector.tensor_copy(stats2h[:, 0:1], mv[:, 0:1])

        # broadcast (mean, rstd) to channels; A = w*rstd, B = b - mean*A
        ab_ps = ps_b.tile([P, 512], F32, tag="psb", name=f"ab{g}")
        for jj in range(2):
            nc.tensor.matmul(
                ab_ps[:, 2 * jj : 2 * jj + 2],
                lhsT=gtmat_sb[0:HG, jj, :],
                rhs=stats2h[:],
                start=True,
                stop=True,
                skip_group_check=True,
            )
        mean_v = ab_ps[:, 0:4].rearrange("p (j two) -> p j two", two=2)[:, :, 0]
        rstd_v = ab_ps[:, 0:4].rearrange("p (j two) -> p j two", two=2)[:, :, 1]
        a_v = ab_sb[:, 2 * g : 2 * g + 2, 0]
        b_v = ab_sb[:, 2 * g : 2 * g + 2, 1]
        nc.vector.tensor_tensor(a_v, gw_sb[:, tj[0] : tj[1] + 1], rstd_v, ALU.mult)
        nc.vector.tensor_tensor(b_v, mean_v, a_v, ALU.mult)
        nc.vector.tensor_tensor(
            b_v, gb_sb[:, tj[0] : tj[1] + 1], b_v, ALU.subtract
        )
        for j in tj:
            nc.scalar.activation(
                xn_sb[:, j, :],
                x_sb[:, j, :],
                FX.Identity,
                bias=ab_sb[:, j, 1:2],
                scale=ab_sb[:, j, 0:1],
            )

    # ---- qk = Wqk @ xn + b --------------------------------------------------
    def qk_unit(m, n):
        def emit():
            qk_ps = ps_b.tile([P, 512], F32, tag="psb")
            for u in range(NCT // 2):
                nc.tensor.matmul(
                    qk_ps[:],
                    lhsT=wqkT_sb[:, 2 * u : 2 * u + 2, m * P : (m + 1) * P],
                    rhs=xn_sb[:, 2 * u : 2 * u + 2, n * 512 : (n + 1) * 512],
                    start=(u == 0),
                    stop=(u == NCT // 2 - 1),
                    perf_mode=DR,
                )
            nc.vector.tensor_scalar_add(
                qk_sb[:, m, n * 512 : (n + 1) * 512], qk_ps[:], bqk_sb[:, m : m + 1]
            )

        return emit

    def emit_qk(m, chunks=range(NC2)):
        for n in chunks:
            qk_unit(m, n)()

    def vt_unit(st):
        return lambda: emit_vt(st)

    def emit_vt(st):
        vt_ps = ps_b.tile([P, 512], F32, tag="psb")
        for u in range(NCT // 2):
            nc.tensor.matmul(
                vt_ps[:],
                lhsT=xn_sb[:, 2 * u : 2 * u + 2, st * P : (st + 1) * P],
                rhs=wvT_sb[:, 2 * u : 2 * u + 2, :],
                start=(u == 0),
                stop=(u == NCT // 2 - 1),
                perf_mode=DR,
            )
        # one copy per st: the dst view [p, pr, q, c] with q sliced at step 3
        # selects exactly the two 64-col v slots (offsets 0 and 192) of each
        # pair block, matching the psum's [p, pr, half, c] layout
        blk = vT_sb[:, st, : NH * HS].rearrange(
            "p (pr q c) -> p pr q c", q=4, c=CH
        )[:, :, 0:4:3, :]
        src = vt_ps[:].rearrange("p (pr half c) -> p pr half c", half=2, c=CH)
        nc.vector.tensor_copy(blk, src)

    def emit_pair_logits(j, wt, fillers=()):
        """Logits + exp for head pair (2j, 2j+1); row-packed K=64 matmuls.
        exp tiles assigned to the DVE (Schraudolph fp8e5 bits) are NOT
        emitted here -- they're returned as a closure the pipeline calls
        after the previous pair's AV drain, so they never block the DVE FIFO
        while waiting on their logits (which would stall PSUM rotation and
        starve ScalarE). The rest run exact exp on ScalarE with direct fp8e5
        output. `fillers` are closures (qk/vT units) emitted one per (st,
        half) iteration from st=1 on -- their PE matmuls slot into the
        psum-rotation idle gaps of the logits stream without delaying it,
        and their drains dribble into the DVE queue in small doses."""
        fillers = list(fillers)
        for st in range(NTT):
            for half in range(2):
                on_dve = _exp_on_dve(j, st, half)
                pool = ps_c if on_dve else ps_a
                lt = pool.tile([P, T], F32, tag="psc" if on_dve else "psa")
                rs = slice(half * CH, (half + 1) * CH)
                for n in range(NC2):
                    nc.tensor.matmul(
                        lt[:, n * 512 : (n + 1) * 512],
                        lhsT=qk_sb[rs, 4 + j, st * P : (st + 1) * P],
                        rhs=qk_sb[rs, j, n * 512 : (n + 1) * 512],
                        start=True,
                        stop=True,
                    )
                wslot = wt[:, st, half * T : (half + 1) * T]
                if on_dve:
                    nc.vector.tensor_scalar(
                        wslot.bitcast(I8), lt[:], SCH_A, SCH_B, ALU.mult, ALU.add
                    )
                else:
                    nc.scalar.activation(wslot, lt[:], FX.Exp, scale=0.125)
                if st >= 2 and fillers:
                    fillers.pop(0)()

    def _proj_mms(pr_ps, m, n):
        # bias seed via K=1 matmul (bp x ones), then fp8 DoubleRow passes
        nc.tensor.matmul(
            pr_ps[:],
            lhsT=bpT_sb[0:1, m * P : (m + 1) * P],
            rhs=onesr_sb[0:1, :],
            start=True,
            stop=False,
            skip_group_check=True,
        )
        for u in range(NCT // 2):
            nc.tensor.matmul(
                pr_ps[:],
                lhsT=wpT_sb[:, 2 * u : 2 * u + 2, m * P : (m + 1) * P],
                rhs=a_sb[:, 2 * u : 2 * u + 2, n * 512 : (n + 1) * 512],
                start=False,
                stop=(u == NCT // 2 - 1),
                perf_mode=DR,
                skip_group_check=True,
            )

    def emit_proj_all():
        # a_sb is fully drained when this runs (the wide AV writes whole-T
        # rows), so both n-chunks are ready at once. Odd m borrows the
        # tail-idle ps_c [P,T] tile for BOTH chunks: one FD=1024 drain and
        # one output DMA instead of two each; even m stays on ps_b [P,512]
        # per chunk -- 3 effective psum buffers deepen the tail pipeline.
        for m in range(NCT):
            # wide units drain on the (tail-idle) ACT queue, small on SP
            eng = nc.scalar if m % 2 == 1 else nc.sync
            if m % 2 == 1:
                pr_wide = ps_c.tile([P, T], F32, tag="psc", name=f"prc{m}")
                for n in range(NC2):
                    _proj_mms(pr_wide[:, n * 512 : (n + 1) * 512], m, n)
                ot = outp.tile([P, T], F32, tag="otw", name=f"otw{m}")
                nc.vector.tensor_add(ot[:], pr_wide[:], x_sb[:, m, :])
                eng.dma_start(out_d[:, m, :], ot[:])
            else:
                for n in range(NC2):
                    pr_ps = ps_b.tile([P, 512], F32, tag="psb")
                    _proj_mms(pr_ps, m, n)
                    ot = outp.tile([P, 512], F32, tag="ot")
                    nc.vector.tensor_add(
                        ot[:], pr_ps[:], x_sb[:, m, n * 512 : (n + 1) * 512]
                    )
                    eng.dma_start(
                        out_d[:, m, n * 512 : (n + 1) * 512], ot[:]
                    )

    def emit_pair_av_wide(j, wt):
        # tail variant: after the last exp the ps_a banks are free, so each
        # half accumulates both n-chunks into one [P, T] ps_a tile and the
        # drain runs at FD=1024 with no n-serialization; ps_b stays free for
        # the proj that follows.
        lo, hi = slice(0, CH), slice(CH, P)
        avs = {}
        for half in (1, 0):
            h = 2 * j + half
            av_ps = ps_a.tile([P, T], F32, tag="psa")
            for n in range(NC2):
                for u in range(NTT // 2):
                    nc.tensor.matmul(
                        av_ps[:, n * 512 : (n + 1) * 512],
                        lhsT=vT_sb[:, 2 * u : 2 * u + 2, h * HS : (h + 1) * HS],
                        rhs=wt[
                            :,
                            2 * u : 2 * u + 2,
                            half * T + n * 512 : half * T + (n + 1) * 512,
                        ],
                        start=(u == 0),
                        stop=(u == NTT // 2 - 1),
                        perf_mode=DR,
                        skip_group_check=True,
                    )
            avs[half] = av_ps
        zs1 = wide.tile([P, T], F32, tag="zsw")
        zi1 = wide.tile([P, T], F32, tag="ziw")
        nc.vector.reciprocal_approx_fast(zi1[lo, :], avs[1][lo, :])
        nc.gpsimd.dma_start(zs1[hi, :], zi1[lo, :])
        zs0 = wide.tile([P, T], F32, tag="zsw")
        zc0 = wide.tile([P, T], F32, tag="zcw")
        nc.vector.tensor_copy(zc0[hi, :], avs[0][hi, :])
        nc.gpsimd.dma_start(zs0[lo, :], zc0[hi, :])
        nc.vector.tensor_tensor(a_sb[hi, j, :], avs[1][hi, :], zs1[hi, :], ALU.mult)
        zi0 = wide.tile([P, T], F32, tag="ziw")
        nc.vector.reciprocal_approx_fast(zi0[lo, :], zs0[lo, :])
        nc.vector.tensor_tensor(a_sb[lo, j, :], avs[0][lo, :], zi0[lo, :], ALU.mult)

    def av_units(j, wt):
        # even head: rows 0..63 = attention, 64..127 = Z (odd: swapped). The
        # custom-DVE reciprocal only works at base partition 0, and engines
        # can't shift data across partitions -- a small SBUF->SBUF DMA does
        # the 64-partition move. Returns 6 closures per pair, interleaved as
        # fillers into the NEXT pair's logits emission so the AV matmuls run
        # in the PE's psum-rotation idle gaps (not after the last exp) and
        # the strict-FIFO DVE never waits on an in-flight shift DMA:
        # A(half, n) = matmuls + DMA-independent op + DMA kick;
        # B(n) = DMA-dependent recip/mults, >=2 filler slots later.
        lo, hi = slice(0, CH), slice(CH, P)
        avs, zss = {}, {}

        def unit_a(half, n):
            def emit():
                h = 2 * j + half
                av_ps = ps_b.tile([P, 512], F32, tag="psb")
                for u in range(NTT // 2):
                    nc.tensor.matmul(
                        av_ps[:],
                        lhsT=vT_sb[:, 2 * u : 2 * u + 2, h * HS : (h + 1) * HS],
                        rhs=wt[
                            :,
                            2 * u : 2 * u + 2,
                            half * T + n * 512 : half * T + (n + 1) * 512,
                        ],
                        start=(u == 0),
                        stop=(u == NTT // 2 - 1),
                        perf_mode=DR,
                    )
                avs[half, n] = av_ps
                zs = small.tile([P, 512], F32, tag="zs")
                zss[half, n] = zs
                if half == 1:
                    zi1 = small.tile([P, 512], F32, tag="zi")
                    nc.vector.reciprocal_approx_fast(zi1[lo, :], av_ps[lo, :])
                    nc.gpsimd.dma_start(zs[hi, :], zi1[lo, :])
                else:
                    zc0 = small.tile([P, 512], F32, tag="zc")
                    nc.vector.tensor_copy(zc0[hi, :], av_ps[hi, :])
                    nc.gpsimd.dma_start(zs[lo, :], zc0[hi, :])

            return emit

        def unit_b(n):
            def emit():
                nc.vector.tensor_tensor(
                    a_sb[hi, j, n * 512 : (n + 1) * 512],
                    avs[1, n][hi, :],
                    zss[1, n][hi, :],
                    ALU.mult,
                )
                zi0 = small.tile([P, 512], F32, tag="zi")
                nc.vector.reciprocal_approx_fast(zi0[lo, :], zss[0, n][lo, :])
                nc.vector.tensor_tensor(
                    a_sb[lo, j, n * 512 : (n + 1) * 512],
                    avs[0, n][lo, :],
                    zi0[lo, :],
                    ALU.mult,
                )

            return emit

        return [unit_a(1, 0), unit_a(0, 0), unit_b(0),
                unit_a(1, 1), unit_a(0, 1), unit_b(1)]

    def emit_pair_av(j, wt):
        for f in av_units(j, wt):
            f()


    def tap_out(src3d):
        for m in range(NCT):
            dt_t = outp.tile([P, T], F32, tag="dbg", name=f"stg{m}")
            nc.vector.tensor_copy(dt_t[:], src3d[:, m, :])
            nc.sync.dma_start(out_d[:, m, :], dt_t[:])

    if _STAGE == 1:
        tap_out(xn_sb)
        return

    # rows 0/4 up front (logits(0) needs them); the remaining qk rows and all
    # vT ride as fillers inside the pair windows: row j+1/5+j inside window j
    # so each pair's logits only ever wait on long-drained rows
    emit_qk(0)
    emit_qk(4)
    pad = lambda: None

    def window_fillers(av, qks):
        # [A,A,pad,B, A,A,pad,B, qk...]: respects the 2-buffer ps_b WAR
        # chain (an A-unit's psum buffer is reused 2 allocs later, so its
        # reader B sits in between) and gives every shift DMA >=2 slots
        # before its dependent B op.
        if av is None:
            return qks
        return [av[0], av[1], pad, av[2], av[3], av[4], pad, av[5]] + qks

    winf0 = (
        [qk_unit(1, n) for n in range(NC2)]
        + [qk_unit(5, n) for n in range(NC2)]
        + [vt_unit(st) for st in range(NTT)]
    )
    wts0 = wt_pool.tile([P, NTT, 2 * T], FP8E5, tag="w", name="wt0")
    emit_pair_logits(0, wts0, winf0)
    if _DEBUG_TAP == "w0":
        for m in range(NCT):
            dt_t = outp.tile([P, T], F32, tag="dbg", name=f"dbg{m}")
            nc.vector.tensor_copy(dt_t[:], wts0[:, m, 0:T])
            nc.sync.dma_start(out_d[:, m, :], dt_t[:])
        return
    if _DEBUG_TAP == "av0":
        emit_pair_av(0, wts0)
        return
    if _STAGE == 2:
        tap_out(qk_sb[:, 0:NCT, :])
        return
    if _STAGE == 3:
        # logits+exp only: tiny reads release the wt slots without AV
        dmy = small.tile([P, 8], F32, tag="dmy")
        nc.vector.tensor_copy(dmy[:, 0:8], wts0[:, 7, 2040:2048])
        for j in range(1, 4):
            wts = wt_pool.tile([P, NTT, 2 * T], FP8E5, tag="w", name=f"wt{j}")
            emit_pair_logits(j, wts)
            nc.vector.tensor_copy(dmy[:, 0:8], wts[:, 7, 2040:2048])
        tap_out(qk_sb[:, 0:NCT, :])
        return
    # One-pair software pipeline: logits(j+1) are emitted (= higher scheduler
    # priority) before AV(j), so ScalarE always has exp input ready while the
    # PE drains the previous pair's AV matmuls in the gaps. The remaining
    # qk row tiles are spread across the pair windows to keep the PE from
    # idling long enough to trip the HAM clock throttle.
    prev = wts0
    for j in range(1, 4):
        wts = wt_pool.tile([P, NTT, 2 * T], FP8E5, tag="w", name=f"wt{j}")
        qks = (
            [qk_unit(j + 1, n) for n in range(NC2)]
            + [qk_unit(5 + j, n) for n in range(NC2)]
            if j < 3
            else []
        )
        emit_pair_logits(j, wts, qks)
        emit_pair_av(j - 1, prev)
        prev = wts
    emit_pair_av_wide(3, prev)
    if _STAGE == 4:
        tap_out(a_sb)
        return

    # optional debug tap: overwrite `out` with an intermediate tensor
    if _DEBUG_TAP == "xn":
        for m in range(NCT):
            dt_t = outp.tile([P, T], F32, tag="dbg", name=f"dbg{m}")
            nc.vector.tensor_copy(dt_t[:], xn_sb[:, m, :])
            nc.sync.dma_start(out_d[:, m, :], dt_t[:])
        return
    if _DEBUG_TAP == "qk":  # q rows: qk tiles 0..3 -> out tiles 0..3
        for m in range(NCT):
            dt_t = outp.tile([P, T], F32, tag="dbg", name=f"dbg{m}")
            nc.vector.tensor_copy(dt_t[:], qk_sb[:, m, :])
            nc.sync.dma_start(out_d[:, m, :], dt_t[:])
        return
    if _DEBUG_TAP == "k":  # k rows: qk tiles 4..7
        for m in range(NCT):
            dt_t = outp.tile([P, T], F32, tag="dbg", name=f"dbg{m}")
            nc.vector.tensor_copy(dt_t[:], qk_sb[:, 4 + m, :])
            nc.sync.dma_start(out_d[:, m, :], dt_t[:])
        return
    if _DEBUG_TAP == "vt":  # vT tiles 0..3 (with ones cols)
        for m in range(NCT):
            dt_t = outp.tile([P, T], F32, tag="dbg", name=f"dbg{m}")
            nc.vector.tensor_copy(dt_t[:], vT_sb[:, m, :])
            nc.sync.dma_start(out_d[:, m, :], dt_t[:])
        return
    if _DEBUG_TAP == "a":
        for m in range(NCT):
            dt_t = outp.tile([P, T], F32, tag="dbg", name=f"dbg{m}")
            nc.vector.tensor_copy(dt_t[:], a_sb[:, m, :])
            nc.sync.dma_start(out_d[:, m, :], dt_t[:])
        return

    # ---- proj + bias + residual --------------------------------------------
    emit_proj_all()


_ACT_SET = "natural_log_exp_and_others"  # contains Square/Ln/Exp/Identity/Copy


def _pin_act_tables():
    """Restrict the ACT table chooser to one set that covers every function
    this kernel uses. Without this, bacc picks each function's 'home' set and
    the GroupNorm Square/Ln/Exp/Identity sequence thrashes ACT_TABLE_LOADs
    (~2.7us each on HW, 5 loads observed). Dict length/order is preserved so
    act_func_set_ids still index act_info.json correctly."""
    import concourse.bacc as _bacc

    orig = _bacc.get_activation_tables

    def patched(arch):
        tabs = orig(arch)
        return {k: (v if k == _ACT_SET else set()) for k, v in tabs.items()}

    _bacc.get_activation_tables = patched
    return lambda: setattr(_bacc, "get_activation_tables", orig)


def build_nc(loop_n=0):
    """loop_n > 0 wraps the body in a For_i running it loop_n times --
    used only by the timing harness to amortize host/RPC overhead."""
    restore = _pin_act_tables()
    try:
        return _build_nc_inner(loop_n)
    finally:
        restore()


def _build_nc_inner(loop_n=0):
    nc = bacc.Bacc("TRN2", target_bir_lowering=False, debug=False)
    io = {}
    io["x"] = nc.dram_tensor("x", [NCT, P, T], F32, kind="ExternalInput").ap()
    io["gw"] = nc.dram_tensor("gw", [P, NCT], F32, kind="ExternalInput").ap()
    io["gb"] = nc.dram_tensor("gb", [P, NCT], F32, kind="ExternalInput").ap()
    io["wqkT"] = nc.dram_tensor("wqkT", [P, NCT, 2 * C], FP8, kind="ExternalInput").ap()
    io["bqk"] = nc.dram_tensor("bqk", [P, NQK], F32, kind="ExternalInput").ap()
    io["wvT"] = nc.dram_tensor("wvT", [P, NCT, C], FP8, kind="ExternalInput").ap()
    io["wpT"] = nc.dram_tensor("wpT", [P, NCT, C], FP8, kind="ExternalInput").ap()
    io["bp"] = nc.dram_tensor("bp", [P, NCT], F32, kind="ExternalInput").ap()
    io["gmat"] = nc.dram_tensor("gmat", [P, 2, 16], F32, kind="ExternalInput").ap()
    io["gtmat"] = nc.dram_tensor("gtmat", [16, 2, P], F32, kind="ExternalInput").ap()
    io["bpT"] = nc.dram_tensor("bpT", [1, C], BF16, kind="ExternalInput").ap()
    io["out"] = nc.dram_tensor("out", [P, NCT, T], F32, kind="ExternalOutput").ap()
    with tile.TileContext(nc) as tc:
        with ExitStack() as ctx:
            if loop_n:
                with tc.For_i(0, loop_n, 1):
                    _build_body(ctx, tc, io)
            else:
                _build_body(ctx, tc, io)
    nc.compile()
    return nc


def _tile_cmaj(a, ntiles):
    """[ntiles*128, F...] -> [128, ntiles, F...] (partition-major tiling)."""
    return np.ascontiguousarray(
        a.reshape(ntiles, P, *a.shape[1:]).swapaxes(0, 1)
    )


def prep_inputs(x, norm_w, norm_b, qkv_w, qkv_b, proj_w, proj_b):
    f = np.float32
    x = np.asarray(x, f)
    norm_w = np.asarray(norm_w, f)
    norm_b = np.asarray(norm_b, f)
    qkv_w = np.asarray(qkv_w, f)
    qkv_b = np.asarray(qkv_b, f)
    proj_w = np.asarray(proj_w, f)
    proj_b = np.asarray(proj_b, f)

    wr = qkv_w.reshape(NH, 3, CH, C)
    Wq = wr[:, 0].reshape(C, C)
    Wk = wr[:, 1].reshape(C, C)
    Wv = wr[:, 2].reshape(C, C)
    br = qkv_b.reshape(NH, 3, CH)
    bq = br[:, 0].reshape(C)
    bk = br[:, 1].reshape(C)
    bv = br[:, 2].reshape(C)

    common = {}
    common["gw"] = _tile_cmaj(norm_w, NCT)
    common["gb"] = _tile_cmaj(norm_b, NCT)
    import concourse.mybir as _mb
    f8 = _mb.dt.np(FP8)

    def to_f8(a):
        return np.clip(a, -240.0, 240.0).astype(f8)

    common["wqkT"] = to_f8(_tile_cmaj(np.concatenate([Wq, Wk], 0).T.copy(), NCT))
    common["bqk"] = _tile_cmaj(np.concatenate([bq, bk]), NQK)
    common["wvT"] = to_f8(_tile_cmaj(Wv.T.copy(), NCT))
    common["wpT"] = to_f8(_tile_cmaj(proj_w.T.copy(), NCT))
    bp_full = proj_b + proj_w @ bv
    common["bp"] = _tile_cmaj(bp_full, NCT)
    import ml_dtypes
    common["bpT"] = bp_full.reshape(1, C).astype(ml_dtypes.bfloat16)

    # per-half group matrices: within a half, tile jj's partitions map to
    # groups 8*jj + p//16 (identical for both halves)
    pidx = np.arange(P)
    gmat = np.zeros((P, 2, 16), f)
    gtmat = np.zeros((16, 2, P), f)
    for jj in range(2):
        grp = 8 * jj + pidx // CPG
        gmat[pidx, jj, grp] = 1.0
        gtmat[grp, jj, pidx] = 1.0
    common["gmat"] = gmat
    common["gtmat"] = gtmat

    in_maps = []
    for b in range(B):
        m = dict(common)
        m["x"] = np.ascontiguousarray(x[b].reshape(NCT, P, T))
        in_maps.append(m)
    return in_maps


_NC_CACHE = []


def _get_nc():
    if not _NC_CACHE:
        _NC_CACHE.append(build_nc())
    return _NC_CACHE[0]


def run(in_maps, trace=False, **kw):
    from concourse.bass_utils import run_bass_kernel_spmd

    nc = _get_nc()
    return run_bass_kernel_spmd(nc, in_maps, list(range(B)), trace=trace, **kw)


def kernel(x, norm_w, norm_b, qkv_w, qkv_b, proj_w, proj_b):
    in_maps = prep_inputs(x, norm_w, norm_b, qkv_w, qkv_b, proj_w, proj_b)
    res = run(in_maps).results
    outs = [
        res[b]["out"].swapaxes(0, 1).reshape(C, 32, 32) for b in range(B)
    ]
    return np.stack(outs).astype(np.float32)


if __name__ == "__main__":
    nc = build_nc()
    print("built ok:", len(nc.m.functions[0].instructions) if hasattr(nc.m.functions[0], "instructions") else "n/a")

